# revision 32
# baseline (speedup 1.0000x reference)
"""Trainium2 Bass kernel for nn_DetectionLoss (B=512, N=252, C=256).

Strategy (pure data parallel over batch, 8 cores x 64 batches):
each core computes partial sums of every loss term over its 64 batches;
the host combines them (all reference reductions are global means).

Key algebraic simplifications (cls = per-batch permutation of 0..N-1,
target[...,4] == cls, target[...,0] == mask m in {0,1}):
  - CE softmax over the box axis n with masked logits m_n*o[b,n,4+j]:
      LSE[b,j] = log( sum_n m_n*exp(o[b,n,4+j]) + (N - cnt_b) )
      picked[b,j] = present*m_j*diag_j + (1-present)*m_0*row0_j - LSE[b,j]
    with diag_j = o[b,j,4+j], row0_j = o[b,0,4+j].
  - the scatter reduces to s_c[j] = sum_i [cls_i==j]*(m*t_c)_i (c=1..3) and
    present[j] = sum_i [cls_i==j]*m_i -> one-hot matmuls on the PE.
  - target channels 5..255 are never used -> read only target[...,0:5].

Implementation notes (perf):
  - big stream output[:, :, 4:256] in n-on-partition layout, exp on ACT in
    8 group-sized ops.
  - reversed matmuls with tiny stationaries (mask column / W [126,4]) and
    256-wide float32r movers (1 cycle/row); outputs land as PSUM rows and
    are flushed PSUM->DRAM, then read back re-laid-out to batch-major.
  - one-hot P built by tensor_tensor(is_equal) against an iota row with the
    cls scalar free-broadcast (never the per-partition scalar PTR mode,
    which runs ~20x slower); builds alternate between DVE and GpSimd.
Output: per-core partial sums [64, 8]; host combines in float64.
"""

import numpy as np

B, N, C = 512, 252, 256
NCORES = 8
NB = B // NCORES          # 64 batches per core
H = N // 2                # 126 partitions (half of N)
GB = 8                    # batches per stream group
NGRP = NB // GB           # 8 groups
BSTRIDE = N * C           # 64512 elements per batch
NP = 256                  # padded mover width (fp32r wants >= 256)

_PROGRAM = None


def _build_program(debug_taps: bool = False):
    import concourse.bass as bass
    import concourse.tile as tile
    from concourse import bacc, mybir
    from concourse.masks import make_identity
    from contextlib import ExitStack

    f32 = mybir.dt.float32
    bf16 = mybir.dt.bfloat16
    i32 = mybir.dt.int32
    Alu = mybir.AluOpType
    Act = mybir.ActivationFunctionType
    X = mybir.AxisListType.X

    nc = bacc.Bacc(
        "TRN2", target_bir_lowering=False, debug=False, num_devices=NCORES
    )
    out_h = nc.dram_tensor("output", [NB, N, C], f32, kind="ExternalInput")
    tgt_h = nc.dram_tensor("target", [NB, N, C], f32, kind="ExternalInput")
    part_h = nc.dram_tensor("partials", [NB, 8], f32, kind="ExternalOutput")
    stageS_h = nc.dram_tensor("stageS", [NB, N], f32, kind="Internal")
    stage4_h = nc.dram_tensor("stage4", [4, NB, N], f32, kind="Internal")
    if debug_taps:
        dbg_sback_h = nc.dram_tensor(
            "dbg_sback", [NB, N * 4], f32, kind="ExternalOutput"
        )
        dbg_ssum_h = nc.dram_tensor("dbg_ssum", [NB, N], f32, kind="ExternalOutput")
        dbg_mt_h = nc.dram_tensor("dbg_mt", [H, 2 * NB], f32, kind="ExternalOutput")

    with tile.TileContext(nc) as tc, ExitStack() as ctx:
        const_pool = ctx.enter_context(tc.tile_pool(name="const", bufs=1))
        sp = ctx.enter_context(tc.tile_pool(name="small", bufs=1))
        stream_pool = ctx.enter_context(tc.tile_pool(name="stream", bufs=3))
        ppool = ctx.enter_context(tc.tile_pool(name="ptile", bufs=32))
        stg_pool = ctx.enter_context(tc.tile_pool(name="stg", bufs=2))
        e_pool = ctx.enter_context(tc.tile_pool(name="epool", bufs=3))

        ident = const_pool.tile([NB, NB], f32)
        make_identity(nc, ident[:])
        iota_i = const_pool.tile([H, NP], i32)
        nc.gpsimd.iota(iota_i[:], pattern=[[1, NP]], base=0, channel_multiplier=0)
        iota_f = const_pool.tile([H, NP], bf16)
        nc.vector.tensor_copy(iota_f[:], iota_i[:])
        iota2 = const_pool.tile([H, 2, NP], bf16)
        nc.vector.tensor_copy(iota2[:, 0, :], iota_i[:])
        nc.vector.tensor_copy(iota2[:, 1, :], iota_i[:])

        # ---- small strided loads (batch on partitions) ----
        t5 = sp.tile([NB, N, 5], f32)
        nc.sync.dma_start(t5[:], tgt_h.ap()[:, :, 0:5])
        o03 = sp.tile([NB, N, 4], f32)
        nc.sync.dma_start(o03[:], out_h.ap()[:, :, 0:4])
        diag = sp.tile([NB, N], f32)
        nc.sync.dma_start(
            diag[:], bass.AP(out_h, 4, [[BSTRIDE, NB], [C + 1, N]])
        )
        row0 = sp.tile([NB, N], f32)
        nc.sync.dma_start(row0[:], bass.AP(out_h, 4, [[BSTRIDE, NB], [1, N]]))

        m_v = t5[:, :, 0]      # mask view [64, 252]
        st0 = stream_pool.tile([H, GB * 2 * C], f32, tag="stream")
        nc.sync.dma_start(
            st0[:].rearrange("p (b h c) -> p b h c", b=GB, h=2, c=C),
            bass.AP(out_h, 0, [[C, H], [BSTRIDE, GB], [H * C, 2], [1, C]]),
        )

        # ---- PE transposes: mask, cls and W columns to n-on-partition ----
        with tc.tile_pool(name="trpsum", bufs=2, space="PSUM") as trp_pool:
            mT = sp.tile([H, 2, NB], bf16)
            cT = sp.tile([H, 2, NB], bf16)
            for h in range(2):
                trp = trp_pool.tile([H, NB], f32, tag="trp")
                nc.tensor.transpose(
                    trp[:], t5[:, h * H:(h + 1) * H, 0], ident[:]
                )
                nc.vector.tensor_copy(mT[:, h, :], trp[:])
                trc = trp_pool.tile([H, NB], f32, tag="trp")
                nc.tensor.transpose(
                    trc[:], t5[:, h * H:(h + 1) * H, 4], ident[:]
                )
                nc.vector.tensor_copy(cT[:, h, :], trc[:])

            # W columns in n-layout: [m*t1, m*t2, m*t3, m]
            mw = sp.tile([NB, N], f32, tag="mw")
            mwT = sp.tile([H, 2, NB, 4], bf16)
            for c in range(1, 4):
                nc.vector.tensor_tensor(mw[:], t5[:, :, c], m_v, op=Alu.mult)
                for h in range(2):
                    trw = trp_pool.tile([H, NB], f32, tag="trp")
                    nc.tensor.transpose(
                        trw[:], mw[:, h * H:(h + 1) * H], ident[:]
                    )
                    nc.vector.tensor_copy(mwT[:, h, :, c - 1], trw[:])
            for h in range(2):
                nc.vector.tensor_copy(mwT[:, h, :, 3], mT[:, h, :])

        if debug_taps:
            mt_f = sp.tile([H, 2 * NB], f32, tag="mtf")
            nc.vector.tensor_copy(mt_f[:], mT[:])
            nc.sync.dma_start(dbg_mt_h.ap()[:], mt_f[:])

        # ---- finale pieces independent of the stream (run early) ----
        cnt = sp.tile([NB, 1], f32)
        nc.vector.reduce_sum(cnt[:], m_v, axis=X)
        kcol = sp.tile([NB, 1], f32)
        nc.vector.tensor_scalar(
            kcol[:], cnt[:], -1.0, float(N), op0=Alu.mult, op1=Alu.add
        )
        lnp = sp.tile([NB, N], f32)
        nc.scalar.activation(lnp[:], o03[:, :, 0], Act.Ln)
        ln1p = sp.tile([NB, N], f32)
        nc.scalar.activation(ln1p[:], o03[:, :, 0], Act.Ln, bias=1.0, scale=-1.0)
        dbce = sp.tile([NB, N], f32)
        nc.vector.tensor_tensor(dbce[:], lnp[:], ln1p[:], op=Alu.subtract)
        nc.vector.tensor_tensor(dbce[:], dbce[:], m_v, op=Alu.mult)
        nc.vector.tensor_tensor(dbce[:], dbce[:], ln1p[:], op=Alu.add)
        bce_row = sp.tile([NB, 1], f32)
        nc.vector.reduce_sum(bce_row[:], dbce[:], axis=X)
        r0m = sp.tile([NB, N], f32)
        nc.vector.tensor_tensor(
            r0m[:], row0[:], t5[:, 0:1, 0].to_broadcast([NB, N]), op=Alu.mult
        )
        premo1 = sp.tile([NB, N], f32)
        nc.vector.tensor_tensor(premo1[:], o03[:, :, 1], m_v, op=Alu.mult)
        premo2 = sp.tile([NB, N], f32)
        nc.vector.tensor_tensor(premo2[:], o03[:, :, 2], m_v, op=Alu.mult)
        mo3 = sp.tile([NB, N], f32)
        nc.vector.tensor_tensor(mo3[:], o03[:, :, 3], m_v, op=Alu.mult)
        mo3_row = sp.tile([NB, 1], f32)
        nc.vector.reduce_sum(mo3_row[:], mo3[:], axis=X)

        # ---- main loop: stream exp + reversed matmuls + PSUM flushes ----
        # S[b, j]  = sum_n m_n * E[b, n, j]        (lhsT = mask column)
        # S4[c, j] = sum_i P[i, j] * W[i, c]       (lhsT = W [126, 4])
        # movers are 256 wide: S reads 4 cols into the next (b,h) block
        # (valid data, cols 252..255 of the output are ignored); the last
        # block reads the memset tail. P is built 256 wide (cols >= 252
        # compare against iota >= 252 and are exactly 0).
        with tc.tile_pool(name="mmpsum", bufs=2, space="PSUM") as mm_pool:
            for g in range(NGRP):
                if g == 0:
                    st = st0
                else:
                    st = stream_pool.tile([H, GB * 2 * C], f32, tag="stream")
                    nc.sync.dma_start(
                        st[:].rearrange(
                            "p (b h c) -> p b h c", b=GB, h=2, c=C
                        ),
                        bass.AP(
                            out_h,
                            g * GB * BSTRIDE,
                            [[C, H], [BSTRIDE, GB], [H * C, 2], [1, C]],
                        ),
                    )
                et = e_pool.tile([H, GB * 2 * C + 4], bf16, tag="et")
                nc.vector.memset(et[:, GB * 2 * C:], 1.0)
                nc.scalar.activation(
                    et[:, 0:GB * 2 * C], st[:], Act.Exp
                )
                # rows 0..3: S4 (s1,s2,s3,present); row 32: S (masked expsum)
                mm = mm_pool.tile([33, GB, NP], f32, tag="mm")
                for k in range(GB):
                    b = g * GB + k
                    P2 = ppool.tile([H, 2, NP], bf16, tag="P2")
                    nc.vector.tensor_tensor(
                        P2[:], iota2[:],
                        bass.AP(
                            cT[:].tensor, cT[:].offset + b,
                            [cT[:].ap[0], [NB, 2], [0, NP]],
                        ),
                        op=Alu.is_equal,
                    )
                    Ps = [P2[:, 0, :], P2[:, 1, :]]
                    for h in range(2):
                        pos = (k * 2 + h) * C + 4
                        nc.tensor.matmul(
                            mm[32:33, k, :],
                            lhsT=mT[:, h, b:b + 1],
                            rhs=et[:, pos:pos + NP],
                            start=(h == 0),
                            stop=(h == 1),
                        )
                    for h in range(2):
                        nc.tensor.matmul(
                            mm[0:4, k, :],
                            lhsT=mwT[:, h, b, :],
                            rhs=Ps[h],
                            start=(h == 0),
                            stop=(h == 1),
                        )
                stg4 = stg_pool.tile([4, GB, N], f32, tag="stg4")
                stgS = stg_pool.tile([1, GB, N], f32, tag="stgS")
                if g % 2 == 0:
                    nc.scalar.copy(stg4[:], mm[0:4, :, 0:N])
                    nc.vector.tensor_copy(stgS[:], mm[32:33, :, 0:N])
                else:
                    nc.vector.tensor_copy(stg4[:], mm[0:4, :, 0:N])
                    nc.scalar.copy(stgS[:], mm[32:33, :, 0:N])
                b0 = g * GB
                nc.sync.dma_start(
                    bass.AP(stageS_h, b0 * N, [[N, GB], [1, N]]),
                    stgS[:],
                )
                nc.scalar.dma_start(
                    bass.AP(
                        stage4_h, b0 * N, [[NB * N, 4], [N, GB], [1, N]]
                    ),
                    stg4[:],
                )

        # ---- read back re-laid-out to batch-major ----
        Ssum = sp.tile([NB, N], f32)
        nc.scalar.dma_start(Ssum[:], stageS_h.ap()[:])
        sb4 = sp.tile([NB, 4, N], f32)
        nc.scalar.dma_start(
            sb4[:], bass.AP(stage4_h, 0, [[N, NB], [NB * N, 4], [1, N]])
        )
        pres_v = sb4[:, 3, :]
        if debug_taps:
            nc.sync.dma_start(dbg_sback_h.ap()[:], sb4[:])
            nc.sync.dma_start(dbg_ssum_h.ap()[:], Ssum[:])

        # ---- tail finale (needs Ssum / sb4) ----
        nc.vector.tensor_tensor(
            Ssum[:], Ssum[:], kcol[:].to_broadcast([NB, N]), op=Alu.add
        )
        lse = sp.tile([NB, N], f32)
        lse_row = sp.tile([NB, 1], f32)
        nc.scalar.activation(lse[:], Ssum[:], Act.Ln, accum_out=lse_row[:])

        # sel: present*m*diag + (1-present)*m0*row0
        selt = sp.tile([NB, N], f32)
        nc.vector.tensor_tensor(selt[:], pres_v, m_v, op=Alu.mult)
        nc.vector.tensor_tensor(selt[:], selt[:], diag[:], op=Alu.mult)
        onep = sp.tile([NB, N], f32)
        nc.vector.tensor_scalar(
            onep[:], pres_v, -1.0, 1.0, op0=Alu.mult, op1=Alu.add
        )
        nc.vector.tensor_tensor(onep[:], onep[:], r0m[:], op=Alu.mult)
        nc.vector.tensor_tensor(selt[:], selt[:], onep[:], op=Alu.add)
        sel_row = sp.tile([NB, 1], f32)
        nc.vector.reduce_sum(sel_row[:], selt[:], axis=X)

        # MSE x/y: sum (m*o_c - s_c)^2
        junk = sp.tile([NB, N], f32)
        mo = sp.tile([NB, N], f32)
        lx_row = sp.tile([NB, 1], f32)
        nc.vector.tensor_tensor(mo[:], premo1[:], sb4[:, 0, :], op=Alu.subtract)
        nc.scalar.activation(junk[:], mo[:], Act.Square, accum_out=lx_row[:])
        ly_row = sp.tile([NB, 1], f32)
        nc.vector.tensor_tensor(mo[:], premo2[:], sb4[:, 1, :], op=Alu.subtract)
        nc.scalar.activation(junk[:], mo[:], Act.Square, accum_out=ly_row[:])

        # MSE wh: sum m*o3 + sum s3 - 2*sum sqrt(m*o3*s3)
        s3_row = sp.tile([NB, 1], f32)
        nc.vector.reduce_sum(s3_row[:], sb4[:, 2, :], axis=X)
        nc.vector.tensor_tensor(mo3[:], mo3[:], sb4[:, 2, :], op=Alu.mult)
        sq3_row = sp.tile([NB, 1], f32)
        nc.scalar.activation(junk[:], mo3[:], Act.Sqrt, accum_out=sq3_row[:])
        lwh_row = sp.tile([NB, 1], f32)
        nc.vector.tensor_tensor(lwh_row[:], mo3_row[:], s3_row[:], op=Alu.add)
        nc.vector.tensor_scalar(
            sq3_row[:], sq3_row[:], -2.0, None, op0=Alu.mult
        )
        nc.vector.tensor_tensor(lwh_row[:], lwh_row[:], sq3_row[:], op=Alu.add)

        # ---- pack partials [64, 8] and store ----
        pt = sp.tile([NB, 8], f32)
        nc.vector.memset(pt[:], 0.0)
        nc.vector.tensor_copy(pt[:, 0:1], bce_row[:])
        nc.vector.tensor_copy(pt[:, 1:2], lse_row[:])
        nc.vector.tensor_copy(pt[:, 2:3], sel_row[:])
        nc.vector.tensor_copy(pt[:, 3:4], lx_row[:])
        nc.vector.tensor_copy(pt[:, 4:5], ly_row[:])
        nc.vector.tensor_copy(pt[:, 5:6], lwh_row[:])
        nc.sync.dma_start(part_h.ap()[:], pt[:])

    nc.compile()
    return nc


def get_program(debug_taps: bool = False):
    global _PROGRAM
    if debug_taps:
        return _build_program(debug_taps=True)
    if _PROGRAM is None:
        _PROGRAM = _build_program()
    return _PROGRAM


def combine_partials(partials: np.ndarray) -> np.float32:
    """partials: [..., 8] summed over all leading axes -> scalar loss."""
    tot = partials.astype(np.float64).reshape(-1, 8).sum(axis=0)
    BN = B * N
    bce = -tot[0] / BN
    ce = (tot[1] - tot[2]) / BN
    mse = tot[3] / BN + tot[4] / BN + 2.0 * (tot[5] / BN)
    return np.float32(10.0 * mse + bce + 0.5 * (1.0 - bce) + ce)


def kernel(output: np.ndarray, target: np.ndarray, _trace=[False]) -> np.ndarray:
    from concourse.bass_utils import run_bass_kernel_spmd

    nc = get_program()
    in_maps = []
    for c in range(NCORES):
        sl = slice(c * NB, (c + 1) * NB)
        in_maps.append(
            {
                "output": np.ascontiguousarray(output[sl], dtype=np.float32),
                "target": np.ascontiguousarray(target[sl], dtype=np.float32),
            }
        )
    res = run_bass_kernel_spmd(
        nc, in_maps, core_ids=list(range(NCORES)), trace=_trace[0]
    )
    partials = np.stack([r["partials"] for r in res.results])
    kernel.last_result = res
    return np.asarray(combine_partials(partials), dtype=np.float32)
